# revision 20
# baseline (speedup 1.0000x reference)
"""Trainium2 Bass kernel: 3x3 'same' cross-correlation on a [1,1,8192,8192] fp32 image.

Strategy (8 NeuronCores, row-sharded, memory-bound target):
  - Host: pad image by 1 row/col, cast to bf16 (tolerance is 2e-2 rel; bf16
    I/O costs ~4e-3), shard into 8 overlapping [1026, 8194] row-shards. The
    image and kernel are flipped vertically when that makes the single-tap
    kernel column's tap land on jmin (legal partition-0 pull, see below).
  - Device (per core): for each tile of R=126 output rows, one [R+nb, 8194]
    bf16 image tile A. Vertical taps are applied with one banded matmul per
    nonzero kernel *column* (PE shifts across partitions; free-dim offsets
    handle the kernel column on the rhs AP; bf16 streams 1 row/cycle vs
    fp16's 1/2). A kernel column holding a single tap at j==jmin is pulled
    out of the PE entirely and fused into the PSUM->SBUF drain as
    scalar_tensor_tensor (out = A_shifted * w + psum) — cuts PE work from 3
    to 2 matmuls per chunk. Drains split 3:1 between DVE (fused STT from
    PSUM) and ACT-copy + GPSIMD-add; output DMAs go out on the ACT HWDGE
    queue, input DMAs on SP.
  - bf16 halves HBM traffic both ways: ~16.8 MB in + 16.8 MB out per core,
    ~94us DMA roofline at 358 GB/s per-core HBM bandwidth.
  - Host: concat shard outputs, unflip, upcast to fp32.
"""

import ml_dtypes
import numpy as np

import concourse.bass as bass
import concourse.mybir as mybir
from concourse import bacc
from concourse import bass_utils
from concourse import tile

H = 8192
W = 8192
N_CORES = 8
RPC = H // N_CORES  # rows per core

BF16 = mybir.dt.bfloat16
NP_BF16 = ml_dtypes.bfloat16
F32 = mybir.dt.float32


def _nonzero_taps(kern3: np.ndarray):
    """[(j, i, w)] for nonzero entries of the 3x3 kernel."""
    return [
        (j, i, float(kern3[j, i]))
        for j in range(kern3.shape[0])
        for i in range(kern3.shape[1])
        if kern3[j, i] != 0.0
    ]


def _band_matrix(col_taps, jmin, k_rows, out_rows):
    """lhsT [k_rows, out_rows] with B[k, p] = w for each (j, w) in col_taps
    where k = p + (j - jmin). matmul computes psum[p,:] = sum_k B[k,p]*A[k,:]."""
    B = np.zeros((k_rows, out_rows), dtype=NP_BF16)
    for j, w in col_taps:
        d = j - jmin
        for p in range(out_rows):
            k = p + d
            if 0 <= k < k_rows:
                B[k, p] = w
    return B


def build_program(kern3: np.ndarray, *, width=W, rpc=RPC,
                  psum_cols=1024, mm_cols=512, a_bufs=4,
                  out_bufs=3, psum_bufs=4, tmp_bufs=4,
                  drain_paths=("dve", "act")):
    """Build the per-core Bass program. Shard layout: S[r] =
    padded_image[core_row0 + r], r in [0, rpc+2); out rows r in [0, rpc).

    The image tile carries one zero column of padding each side so every
    tap's rhs slice [c0+i, c0+i+mm_cols) is in range and full-width."""
    taps = _nonzero_taps(kern3)
    assert taps, "all-zero kernel handled host-side"

    jmin = min(j for j, _, _ in taps)
    jmax = max(j for j, _, _ in taps)
    nb = jmax - jmin  # extra rows of A needed beyond R
    R = (128 - nb) & ~1  # output rows per tile (even)

    # group taps by kernel column
    cols = {}
    for j, i, w in taps:
        cols.setdefault(i, []).append((j, w))
    all_col_ids = sorted(cols.keys())

    # Pull one tap into the DVE drain when it erases a whole matmul column.
    # Engines can only address SBUF starting at partition 0, so the pulled
    # tap must sit at j == jmin (partition-aligned with the out tile).
    # ACT-drained chunks can't fuse the tap (no scalar_tensor_tensor on
    # ACT), so they run all three bands on the PE instead.
    pulled = None
    single = sorted(i for i, ts in cols.items()
                    if len(ts) == 1 and ts[0][0] == jmin)
    if single:
        pi = single[0]
        pj, pw = cols[pi][0]
        pulled = (pj, pi, pw)
    col_ids = sorted(i for i in cols if pulled is None or i != pulled[1])

    nc = bacc.Bacc("TRN2", target_bir_lowering=False, debug=False,
                   num_devices=N_CORES)
    s_in = nc.dram_tensor("shard", [rpc + 2, width + 2], BF16, kind="ExternalInput").ap()
    out_d = nc.dram_tensor("out", [rpc, width], BF16, kind="ExternalOutput").ap()
    bands_in = None
    if col_ids:
        bands_in = nc.dram_tensor(
            "bands", [len(all_col_ids), 128, 128], BF16, kind="ExternalInput"
        ).ap()

    # tiles of output rows
    tiles = []
    t = 0
    while t < rpc:
        r = min(R, rpc - t)
        tiles.append((t, r))
        t += r

    wp = width + 2  # padded tile width
    out_dma_eng = nc.scalar if mybir.EngineType.Activation in nc.hwdge_engines else nc.sync

    with tile.TileContext(nc) as tc:
        with (
            tc.tile_pool(name="bandp", bufs=1) as bandp,
            tc.tile_pool(name="ap", bufs=a_bufs) as apool,
            tc.tile_pool(name="op", bufs=out_bufs) as opool,
            tc.tile_pool(name="pp", bufs=psum_bufs, space="PSUM") as ppool,
        ):
            band_tiles = {}
            for ci, i in enumerate(all_col_ids):
                bt = bandp.tile([128, 128], BF16, tag=f"band{ci}")
                nc.sync.dma_start(out=bt, in_=bands_in[ci])
                band_tiles[i] = bt

            n_q = (width + psum_cols - 1) // psum_cols
            eng_i = 0
            for (t0, rt) in tiles:
                krows = rt + nb  # contraction rows for this tile
                a = apool.tile([128, wp], BF16, tag="a")
                nc.sync.dma_start(
                    out=a[0:krows, :],
                    in_=s_in[t0 + jmin: t0 + jmin + krows, :],
                )
                o = opool.tile([128, width], BF16, tag="o")

                for q in range(n_q):
                    q0 = q * psum_cols
                    q1 = min(q0 + psum_cols, width)
                    path = drain_paths[eng_i % len(drain_paths)]
                    eng_i += 1
                    # dve chunks: 2 bands + DVE STT drain fusing the pulled
                    # tap; act chunks: all 3 bands + plain ACT copy drain.
                    # Both drain engines retire chunks faster than the PE
                    # produces them, so the PE never stalls (stalls would
                    # drop it out of the 2.4 GHz p-state).
                    fuse = pulled is not None and path == "dve"
                    mm_cols_set = col_ids if fuse else all_col_ids
                    if mm_cols_set:
                        ps = ppool.tile([128, psum_cols], F32, tag="ps")
                        # band-major: the PE reloads weights once per band
                        # per chunk. Pieces sit in distinct PSUM banks, so
                        # the interleaved accumulation groups are
                        # address-disjoint.
                        for ii, i in enumerate(mm_cols_set):
                            for c0 in range(q0, q1, mm_cols):
                                c1 = min(c0 + mm_cols, q1)
                                # rhs cols [c0+i, c1+i) in padded coords
                                nc.tensor.matmul(
                                    out=ps[0:rt, c0 - q0:c1 - q0],
                                    lhsT=band_tiles[i][0:krows, 0:rt],
                                    rhs=a[0:krows, c0 + i:c1 + i],
                                    start=(ii == 0),
                                    stop=(ii == len(mm_cols_set) - 1),
                                    skip_group_check=True,
                                )
                        if fuse:
                            # out[p, x] = A[p+pj, x+pi]*pw + psum[p, x]
                            pj, pi, pw = pulled
                            nc.vector.scalar_tensor_tensor(
                                out=o[0:rt, q0:q1],
                                in0=a[pj:pj + rt, q0 + pi:q1 + pi],
                                scalar=pw,
                                in1=ps[0:rt, 0:q1 - q0],
                                op0=mybir.AluOpType.mult,
                                op1=mybir.AluOpType.add,
                            )
                        elif path == "dve":
                            nc.vector.tensor_scalar_mul(
                                o[0:rt, q0:q1], ps[0:rt, 0:q1 - q0], 1.0
                            )
                        else:
                            nc.scalar.copy(
                                out=o[0:rt, q0:q1], in_=ps[0:rt, 0:q1 - q0]
                            )
                    else:
                        # single-tap kernel: no matmul at all
                        pj, pi, pw = pulled
                        eng = nc.vector if path == "dve" else nc.gpsimd
                        eng.tensor_scalar_mul(
                            o[0:rt, q0:q1], a[pj:pj + rt, q0 + pi:q1 + pi], pw
                        )
                out_dma_eng.dma_start(out=out_d[t0: t0 + rt, :], in_=o[0:rt, :])

    nc.compile()

    meta = {
        "bands": (
            np.stack([
                _band_matrix(cols[i], jmin, 128, 128) for i in all_col_ids
            ]) if col_ids else None
        ),
    }
    return nc, meta


def _pull_quality(kern3: np.ndarray) -> int:
    """1 if this orientation admits a legal pulled tap (a single-tap kernel
    column whose tap sits at jmin), else 0."""
    taps = _nonzero_taps(kern3)
    if not taps:
        return 0
    jmin = min(j for j, _, _ in taps)
    cols = {}
    for j, i, w in taps:
        cols.setdefault(i, []).append((j, w))
    return int(any(len(ts) == 1 and ts[0][0] == jmin for ts in cols.values()))


_CACHE = {}


def _get_program(kern3: np.ndarray):
    """Pick the vertical orientation (normal / row-flipped) that lets a
    matmul column be pulled into the drain, then build + compile."""
    key = kern3.astype(np.float32).tobytes()
    if key not in _CACHE:
        flip = _pull_quality(kern3[::-1]) > _pull_quality(kern3)
        kern_o = kern3[::-1] if flip else kern3
        nc, meta = build_program(np.ascontiguousarray(kern_o))
        _CACHE[key] = (nc, meta, flip)
    return _CACHE[key]


def kernel(image: np.ndarray, kernel: np.ndarray) -> np.ndarray:
    image = np.asarray(image)
    kernel = np.asarray(kernel, dtype=np.float32)
    img = image.reshape(H, W)

    if not np.any(kernel):
        return np.zeros_like(image, dtype=np.float32).reshape(image.shape)

    nc, meta, flip = _get_program(kernel)

    padded = np.zeros((H + 2, W + 2), dtype=NP_BF16)
    padded[1:-1, 1:-1] = img  # cast fp32 -> fp16
    if flip:
        padded = padded[::-1]  # row-flip view; corr with flipped kernel
    in_maps = []
    for c in range(N_CORES):
        m = {"shard": np.ascontiguousarray(padded[c * RPC: c * RPC + RPC + 2])}
        if meta["bands"] is not None:
            m["bands"] = meta["bands"]
        in_maps.append(m)

    res = bass_utils.run_bass_kernel_spmd(nc, in_maps, core_ids=list(range(N_CORES)))
    out = np.concatenate([r["out"] for r in res.results], axis=0)
    if flip:
        out = out[::-1]
    return np.ascontiguousarray(out).astype(np.float32).reshape(image.shape)


# revision 24
# speedup vs baseline: 1.0113x; 1.0113x over previous
"""Trainium2 Bass kernel: 3x3 'same' cross-correlation on a [1,1,8192,8192] fp32 image.

Strategy (8 NeuronCores, row-sharded, memory-bound target):
  - Host: pad image by 1 row/col, cast to bf16 (tolerance is 2e-2 rel; bf16
    I/O costs ~4e-3), shard into 8 overlapping [1026, 8194] row-shards. The
    image and kernel are flipped vertically when that makes the single-tap
    kernel column's tap land on jmin (legal partition-0 pull, see below).
  - Device (per core): for each tile of R=126 output rows, one [R+nb, 8194]
    bf16 image tile A. Vertical taps are applied with one banded matmul per
    nonzero kernel *column* (PE shifts across partitions; free-dim offsets
    handle the kernel column on the rhs AP; bf16 streams 1 row/cycle vs
    fp16's 1/2). A kernel column holding a single tap at j==jmin is pulled
    out of the PE entirely and fused into the PSUM->SBUF drain as
    scalar_tensor_tensor (out = A_shifted * w + psum) — cuts PE work from 3
    to 2 matmuls per chunk. Drains split 3:1 between DVE (fused STT from
    PSUM) and ACT-copy + GPSIMD-add; output DMAs go out on the ACT HWDGE
    queue, input DMAs on SP.
  - bf16 halves HBM traffic both ways: ~16.8 MB in + 16.8 MB out per core,
    ~94us DMA roofline at 358 GB/s per-core HBM bandwidth.
  - Host: concat shard outputs, unflip, upcast to fp32.
"""

import ml_dtypes
import numpy as np

import concourse.bass as bass
import concourse.mybir as mybir
from concourse import bacc
from concourse import bass_utils
from concourse import tile

H = 8192
W = 8192
N_CORES = 8
RPC = H // N_CORES  # rows per core

BF16 = mybir.dt.bfloat16
NP_BF16 = ml_dtypes.bfloat16
F32 = mybir.dt.float32


def _nonzero_taps(kern3: np.ndarray):
    """[(j, i, w)] for nonzero entries of the 3x3 kernel."""
    return [
        (j, i, float(kern3[j, i]))
        for j in range(kern3.shape[0])
        for i in range(kern3.shape[1])
        if kern3[j, i] != 0.0
    ]


def _band_matrix(col_taps, jmin, k_rows, out_rows):
    """lhsT [k_rows, out_rows] with B[k, p] = w for each (j, w) in col_taps
    where k = p + (j - jmin). matmul computes psum[p,:] = sum_k B[k,p]*A[k,:]."""
    B = np.zeros((k_rows, out_rows), dtype=NP_BF16)
    for j, w in col_taps:
        d = j - jmin
        for p in range(out_rows):
            k = p + d
            if 0 <= k < k_rows:
                B[k, p] = w
    return B


def build_program(kern3: np.ndarray, *, width=W, rpc=RPC,
                  psum_cols=1024, mm_cols=512, a_bufs=4,
                  out_bufs=3, psum_bufs=4, tmp_bufs=4,
                  drain_paths=("dve", "actg", "dve", "act3")):
    """Build the per-core Bass program. Shard layout: S[r] =
    padded_image[core_row0 + r], r in [0, rpc+2); out rows r in [0, rpc).

    The image tile carries one zero column of padding each side so every
    tap's rhs slice [c0+i, c0+i+mm_cols) is in range and full-width."""
    taps = _nonzero_taps(kern3)
    assert taps, "all-zero kernel handled host-side"

    jmin = min(j for j, _, _ in taps)
    jmax = max(j for j, _, _ in taps)
    nb = jmax - jmin  # extra rows of A needed beyond R
    R = (128 - nb) & ~1  # output rows per tile (even)

    # group taps by kernel column
    cols = {}
    for j, i, w in taps:
        cols.setdefault(i, []).append((j, w))
    all_col_ids = sorted(cols.keys())

    # Pull one tap into the DVE drain when it erases a whole matmul column.
    # Engines can only address SBUF starting at partition 0, so the pulled
    # tap must sit at j == jmin (partition-aligned with the out tile).
    # ACT-drained chunks can't fuse the tap (no scalar_tensor_tensor on
    # ACT), so they run all three bands on the PE instead.
    pulled = None
    single = sorted(i for i, ts in cols.items()
                    if len(ts) == 1 and ts[0][0] == jmin)
    if single:
        pi = single[0]
        pj, pw = cols[pi][0]
        pulled = (pj, pi, pw)
    col_ids = sorted(i for i in cols if pulled is None or i != pulled[1])

    nc = bacc.Bacc("TRN2", target_bir_lowering=False, debug=False,
                   num_devices=N_CORES)
    s_in = nc.dram_tensor("shard", [rpc + 2, width + 2], BF16, kind="ExternalInput").ap()
    out_d = nc.dram_tensor("out", [rpc, width], BF16, kind="ExternalOutput").ap()
    bands_in = None
    if col_ids:
        bands_in = nc.dram_tensor(
            "bands", [len(all_col_ids), 128, 128], BF16, kind="ExternalInput"
        ).ap()

    # tiles of output rows
    tiles = []
    t = 0
    while t < rpc:
        r = min(R, rpc - t)
        tiles.append((t, r))
        t += r

    wp = width + 2  # padded tile width
    out_dma_eng = nc.scalar if mybir.EngineType.Activation in nc.hwdge_engines else nc.sync

    with tile.TileContext(nc) as tc:
        with (
            tc.tile_pool(name="bandp", bufs=1) as bandp,
            tc.tile_pool(name="ap", bufs=a_bufs) as apool,
            tc.tile_pool(name="op", bufs=out_bufs) as opool,
            tc.tile_pool(name="tp", bufs=tmp_bufs) as tpool,
            tc.tile_pool(name="pp", bufs=psum_bufs, space="PSUM") as ppool,
        ):
            band_tiles = {}
            for ci, i in enumerate(all_col_ids):
                bt = bandp.tile([128, 128], BF16, tag=f"band{ci}")
                nc.sync.dma_start(out=bt, in_=bands_in[ci])
                band_tiles[i] = bt

            n_q = (width + psum_cols - 1) // psum_cols
            eng_i = 0
            for (t0, rt) in tiles:
                krows = rt + nb  # contraction rows for this tile
                a = apool.tile([128, wp], BF16, tag="a")
                nc.sync.dma_start(
                    out=a[0:krows, :],
                    in_=s_in[t0 + jmin: t0 + jmin + krows, :],
                )
                o = opool.tile([128, width], BF16, tag="o")

                for q in range(n_q):
                    q0 = q * psum_cols
                    q1 = min(q0 + psum_cols, width)
                    path = drain_paths[eng_i % len(drain_paths)]
                    eng_i += 1
                    # Three chunk flavors, spread so every drain engine
                    # retires its chunks faster than the PE produces them
                    # (a PE stall drops it out of the 2.4 GHz p-state):
                    #   dve:  2 bands + DVE STT drain fusing the pulled tap
                    #   act3: all 3 bands + plain ACT copy drain
                    #   actg: 2 bands + ACT copy to tmp + GPSIMD tap add
                    if pulled is None or (path == "actg" and pulled[2] != 1.0):
                        path = "act3" if path != "dve" else "dve"
                    fuse = pulled is not None and path == "dve"
                    skip_pull = fuse or (pulled is not None and path == "actg")
                    mm_cols_set = col_ids if skip_pull else all_col_ids
                    if mm_cols_set:
                        ps = ppool.tile([128, psum_cols], F32, tag="ps")
                        # band-major: the PE reloads weights once per band
                        # per chunk. Pieces sit in distinct PSUM banks, so
                        # the interleaved accumulation groups are
                        # address-disjoint.
                        for ii, i in enumerate(mm_cols_set):
                            for c0 in range(q0, q1, mm_cols):
                                c1 = min(c0 + mm_cols, q1)
                                # rhs cols [c0+i, c1+i) in padded coords
                                nc.tensor.matmul(
                                    out=ps[0:rt, c0 - q0:c1 - q0],
                                    lhsT=band_tiles[i][0:krows, 0:rt],
                                    rhs=a[0:krows, c0 + i:c1 + i],
                                    start=(ii == 0),
                                    stop=(ii == len(mm_cols_set) - 1),
                                    skip_group_check=True,
                                )
                        if fuse:
                            # out[p, x] = A[p+pj, x+pi]*pw + psum[p, x]
                            pj, pi, pw = pulled
                            nc.vector.scalar_tensor_tensor(
                                out=o[0:rt, q0:q1],
                                in0=a[pj:pj + rt, q0 + pi:q1 + pi],
                                scalar=pw,
                                in1=ps[0:rt, 0:q1 - q0],
                                op0=mybir.AluOpType.mult,
                                op1=mybir.AluOpType.add,
                            )
                        elif path == "dve":
                            nc.vector.tensor_scalar_mul(
                                o[0:rt, q0:q1], ps[0:rt, 0:q1 - q0], 1.0
                            )
                        elif path == "actg":
                            pj, pi, pw = pulled
                            tmp = tpool.tile([128, psum_cols], BF16, tag="tmp")
                            nc.scalar.copy(
                                out=tmp[0:rt, 0:q1 - q0], in_=ps[0:rt, 0:q1 - q0]
                            )
                            nc.gpsimd.tensor_add(
                                o[0:rt, q0:q1],
                                a[pj:pj + rt, q0 + pi:q1 + pi],
                                tmp[0:rt, 0:q1 - q0],
                            )
                        else:
                            nc.scalar.copy(
                                out=o[0:rt, q0:q1], in_=ps[0:rt, 0:q1 - q0]
                            )
                    else:
                        # single-tap kernel: no matmul at all
                        pj, pi, pw = pulled
                        eng = nc.vector if path == "dve" else nc.gpsimd
                        eng.tensor_scalar_mul(
                            o[0:rt, q0:q1], a[pj:pj + rt, q0 + pi:q1 + pi], pw
                        )
                out_dma_eng.dma_start(out=out_d[t0: t0 + rt, :], in_=o[0:rt, :])

    nc.compile()

    meta = {
        "bands": (
            np.stack([
                _band_matrix(cols[i], jmin, 128, 128) for i in all_col_ids
            ]) if col_ids else None
        ),
    }
    return nc, meta


def _pull_quality(kern3: np.ndarray) -> int:
    """1 if this orientation admits a legal pulled tap (a single-tap kernel
    column whose tap sits at jmin), else 0."""
    taps = _nonzero_taps(kern3)
    if not taps:
        return 0
    jmin = min(j for j, _, _ in taps)
    cols = {}
    for j, i, w in taps:
        cols.setdefault(i, []).append((j, w))
    return int(any(len(ts) == 1 and ts[0][0] == jmin for ts in cols.values()))


_CACHE = {}


def _get_program(kern3: np.ndarray):
    """Pick the vertical orientation (normal / row-flipped) that lets a
    matmul column be pulled into the drain, then build + compile."""
    key = kern3.astype(np.float32).tobytes()
    if key not in _CACHE:
        flip = _pull_quality(kern3[::-1]) > _pull_quality(kern3)
        kern_o = kern3[::-1] if flip else kern3
        nc, meta = build_program(np.ascontiguousarray(kern_o))
        _CACHE[key] = (nc, meta, flip)
    return _CACHE[key]


def kernel(image: np.ndarray, kernel: np.ndarray) -> np.ndarray:
    image = np.asarray(image)
    kernel = np.asarray(kernel, dtype=np.float32)
    img = image.reshape(H, W)

    if not np.any(kernel):
        return np.zeros_like(image, dtype=np.float32).reshape(image.shape)

    nc, meta, flip = _get_program(kernel)

    padded = np.zeros((H + 2, W + 2), dtype=NP_BF16)
    padded[1:-1, 1:-1] = img  # cast fp32 -> fp16
    if flip:
        padded = padded[::-1]  # row-flip view; corr with flipped kernel
    in_maps = []
    for c in range(N_CORES):
        m = {"shard": np.ascontiguousarray(padded[c * RPC: c * RPC + RPC + 2])}
        if meta["bands"] is not None:
            m["bands"] = meta["bands"]
        in_maps.append(m)

    res = bass_utils.run_bass_kernel_spmd(nc, in_maps, core_ids=list(range(N_CORES)))
    out = np.concatenate([r["out"] for r in res.results], axis=0)
    if flip:
        out = out[::-1]
    return np.ascontiguousarray(out).astype(np.float32).reshape(image.shape)


# revision 27
# speedup vs baseline: 1.0579x; 1.0461x over previous
"""Trainium2 Bass kernel: 3x3 'same' cross-correlation on a [1,1,8192,8192] fp32 image.

Strategy (8 NeuronCores, row-sharded, memory-bound target):
  - Host: pad image by 1 row/col, cast to bf16 (tolerance is 2e-2 rel; bf16
    I/O costs ~4e-3), shard into 8 overlapping [1026, 8194] row-shards. The
    image and kernel are flipped vertically when that makes the single-tap
    kernel column's tap land on jmin (legal partition-0 pull, see below).
  - Device (per core): for each tile of R=126 output rows, one [R+nb, 8194]
    bf16 image tile A. Vertical taps are applied with one banded matmul per
    nonzero kernel *column* (PE shifts across partitions; free-dim offsets
    handle the kernel column on the rhs AP; bf16 streams 1 row/cycle vs
    fp16's 1/2). A kernel column holding a single tap at j==jmin is pulled
    out of the PE entirely and fused into the PSUM->SBUF drain as
    scalar_tensor_tensor (out = A_shifted * w + psum) — cuts PE work from 3
    to 2 matmuls per chunk. Drains split 3:1 between DVE (fused STT from
    PSUM) and ACT-copy + GPSIMD-add; output DMAs go out on the ACT HWDGE
    queue, input DMAs on SP.
  - bf16 halves HBM traffic both ways: ~16.8 MB in + 16.8 MB out per core,
    ~94us DMA roofline at 358 GB/s per-core HBM bandwidth.
  - Host: concat shard outputs, unflip, upcast to fp32.
"""

import ml_dtypes
import numpy as np

import concourse.bass as bass
import concourse.mybir as mybir
from concourse import bacc
from concourse import bass_utils
from concourse import tile

H = 8192
W = 8192
N_CORES = 8
RPC = H // N_CORES  # rows per core

BF16 = mybir.dt.bfloat16
NP_BF16 = ml_dtypes.bfloat16
F32 = mybir.dt.float32


def _nonzero_taps(kern3: np.ndarray):
    """[(j, i, w)] for nonzero entries of the 3x3 kernel."""
    return [
        (j, i, float(kern3[j, i]))
        for j in range(kern3.shape[0])
        for i in range(kern3.shape[1])
        if kern3[j, i] != 0.0
    ]


def _band_matrix(col_taps, jmin, k_rows, out_rows):
    """lhsT [k_rows, out_rows] with B[k, p] = w for each (j, w) in col_taps
    where k = p + (j - jmin). matmul computes psum[p,:] = sum_k B[k,p]*A[k,:]."""
    B = np.zeros((k_rows, out_rows), dtype=NP_BF16)
    for j, w in col_taps:
        d = j - jmin
        for p in range(out_rows):
            k = p + d
            if 0 <= k < k_rows:
                B[k, p] = w
    return B


def build_program(kern3: np.ndarray, *, width=W, rpc=RPC,
                  psum_cols=1024, mm_cols=512, a_bufs=4,
                  out_bufs=3, psum_bufs=4, tmp_bufs=4,
                  drain_paths=("dve", "actg", "dve", "act3")):
    """Build the per-core Bass program. Shard layout: S[r] =
    padded_image[core_row0 + r], r in [0, rpc+2); out rows r in [0, rpc).

    The image tile carries one zero column of padding each side so every
    tap's rhs slice [c0+i, c0+i+mm_cols) is in range and full-width."""
    taps = _nonzero_taps(kern3)
    assert taps, "all-zero kernel handled host-side"

    jmin = min(j for j, _, _ in taps)
    jmax = max(j for j, _, _ in taps)
    nb = jmax - jmin  # extra rows of A needed beyond R
    R = (128 - nb) & ~1  # output rows per tile (even)

    # group taps by kernel column
    cols = {}
    for j, i, w in taps:
        cols.setdefault(i, []).append((j, w))
    all_col_ids = sorted(cols.keys())

    # Pull one tap into the DVE drain when it erases a whole matmul column.
    # Engines can only address SBUF starting at partition 0, so the pulled
    # tap must sit at j == jmin (partition-aligned with the out tile).
    # ACT-drained chunks can't fuse the tap (no scalar_tensor_tensor on
    # ACT), so they run all three bands on the PE instead.
    pulled = None
    single = sorted(i for i, ts in cols.items()
                    if len(ts) == 1 and ts[0][0] == jmin)
    if single:
        pi = single[0]
        pj, pw = cols[pi][0]
        pulled = (pj, pi, pw)
    col_ids = sorted(i for i in cols if pulled is None or i != pulled[1])

    nc = bacc.Bacc("TRN2", target_bir_lowering=False, debug=False,
                   num_devices=N_CORES)
    s_in = nc.dram_tensor("shard", [rpc + 2, width + 2], BF16, kind="ExternalInput").ap()
    out_d = nc.dram_tensor("out", [rpc, width], BF16, kind="ExternalOutput").ap()
    bands_in = None
    if col_ids:
        bands_in = nc.dram_tensor(
            "bands", [len(all_col_ids), 128, 128], BF16, kind="ExternalInput"
        ).ap()

    # tiles of output rows; a short first tile lets compute start before a
    # full-height image load would finish (shorter pipeline fill), and the
    # natural remainder tile at the end shortens the drain->DMA tail.
    tiles = []
    t = 0
    first = min(64, rpc)
    tiles.append((0, first))
    t = first
    while t < rpc:
        r = min(R, rpc - t)
        tiles.append((t, r))
        t += r

    wp = width + 2  # padded tile width
    out_dma_eng = nc.scalar if mybir.EngineType.Activation in nc.hwdge_engines else nc.sync

    with tile.TileContext(nc) as tc:
        with (
            tc.tile_pool(name="bandp", bufs=1) as bandp,
            tc.tile_pool(name="ap", bufs=a_bufs) as apool,
            tc.tile_pool(name="op", bufs=out_bufs) as opool,
            tc.tile_pool(name="tp", bufs=tmp_bufs) as tpool,
            tc.tile_pool(name="pp", bufs=psum_bufs, space="PSUM") as ppool,
        ):
            band_tiles = {}
            for ci, i in enumerate(all_col_ids):
                bt = bandp.tile([128, 128], BF16, tag=f"band{ci}")
                nc.sync.dma_start(out=bt, in_=bands_in[ci])
                band_tiles[i] = bt

            n_q = (width + psum_cols - 1) // psum_cols
            eng_i = 0
            half = (width // 2) + 2  # column split point for the input load
            for (t0, rt) in tiles:
                krows = rt + nb  # contraction rows for this tile
                a = apool.tile([128, wp], BF16, tag="a")
                # two column-half loads: chunks in the left half of the tile
                # only wait on the first one (finer DMA/compute interleave)
                nc.sync.dma_start(
                    out=a[0:krows, 0:half],
                    in_=s_in[t0 + jmin: t0 + jmin + krows, 0:half],
                )
                nc.sync.dma_start(
                    out=a[0:krows, half:wp],
                    in_=s_in[t0 + jmin: t0 + jmin + krows, half:wp],
                )
                o = opool.tile([128, width], BF16, tag="o")

                for q in range(n_q):
                    q0 = q * psum_cols
                    q1 = min(q0 + psum_cols, width)
                    path = drain_paths[eng_i % len(drain_paths)]
                    eng_i += 1
                    # Three chunk flavors, spread so every drain engine
                    # retires its chunks faster than the PE produces them
                    # (a PE stall drops it out of the 2.4 GHz p-state):
                    #   dve:  2 bands + DVE STT drain fusing the pulled tap
                    #   act3: all 3 bands + plain ACT copy drain
                    #   actg: 2 bands + ACT copy to tmp + GPSIMD tap add
                    if pulled is None or (path == "actg" and pulled[2] != 1.0):
                        path = "act3" if path != "dve" else "dve"
                    fuse = pulled is not None and path == "dve"
                    skip_pull = fuse or (pulled is not None and path == "actg")
                    mm_cols_set = col_ids if skip_pull else all_col_ids
                    if mm_cols_set:
                        ps = ppool.tile([128, psum_cols], F32, tag="ps")
                        # band-major: the PE reloads weights once per band
                        # per chunk. Pieces sit in distinct PSUM banks, so
                        # the interleaved accumulation groups are
                        # address-disjoint.
                        for ii, i in enumerate(mm_cols_set):
                            for c0 in range(q0, q1, mm_cols):
                                c1 = min(c0 + mm_cols, q1)
                                # rhs cols [c0+i, c1+i) in padded coords
                                nc.tensor.matmul(
                                    out=ps[0:rt, c0 - q0:c1 - q0],
                                    lhsT=band_tiles[i][0:krows, 0:rt],
                                    rhs=a[0:krows, c0 + i:c1 + i],
                                    start=(ii == 0),
                                    stop=(ii == len(mm_cols_set) - 1),
                                    skip_group_check=True,
                                )
                        if fuse:
                            # out[p, x] = A[p+pj, x+pi]*pw + psum[p, x]
                            pj, pi, pw = pulled
                            nc.vector.scalar_tensor_tensor(
                                out=o[0:rt, q0:q1],
                                in0=a[pj:pj + rt, q0 + pi:q1 + pi],
                                scalar=pw,
                                in1=ps[0:rt, 0:q1 - q0],
                                op0=mybir.AluOpType.mult,
                                op1=mybir.AluOpType.add,
                            )
                        elif path == "dve":
                            nc.vector.tensor_scalar_mul(
                                o[0:rt, q0:q1], ps[0:rt, 0:q1 - q0], 1.0
                            )
                        elif path == "actg":
                            pj, pi, pw = pulled
                            tmp = tpool.tile([128, psum_cols], BF16, tag="tmp")
                            nc.scalar.copy(
                                out=tmp[0:rt, 0:q1 - q0], in_=ps[0:rt, 0:q1 - q0]
                            )
                            nc.gpsimd.tensor_add(
                                o[0:rt, q0:q1],
                                a[pj:pj + rt, q0 + pi:q1 + pi],
                                tmp[0:rt, 0:q1 - q0],
                            )
                        else:
                            nc.scalar.copy(
                                out=o[0:rt, q0:q1], in_=ps[0:rt, 0:q1 - q0]
                            )
                    else:
                        # single-tap kernel: no matmul at all
                        pj, pi, pw = pulled
                        eng = nc.vector if path == "dve" else nc.gpsimd
                        eng.tensor_scalar_mul(
                            o[0:rt, q0:q1], a[pj:pj + rt, q0 + pi:q1 + pi], pw
                        )
                    # store each output column-half as soon as its chunks
                    # have drained — halves the drain->DMA tail latency
                    if q == n_q // 2 - 1:
                        out_dma_eng.dma_start(
                            out=out_d[t0: t0 + rt, 0:q1], in_=o[0:rt, 0:q1]
                        )
                    elif q == n_q - 1:
                        h0 = (n_q // 2) * psum_cols
                        out_dma_eng.dma_start(
                            out=out_d[t0: t0 + rt, h0:q1], in_=o[0:rt, h0:q1]
                        )

    nc.compile()

    meta = {
        "bands": (
            np.stack([
                _band_matrix(cols[i], jmin, 128, 128) for i in all_col_ids
            ]) if col_ids else None
        ),
    }
    return nc, meta


def _pull_quality(kern3: np.ndarray) -> int:
    """1 if this orientation admits a legal pulled tap (a single-tap kernel
    column whose tap sits at jmin), else 0."""
    taps = _nonzero_taps(kern3)
    if not taps:
        return 0
    jmin = min(j for j, _, _ in taps)
    cols = {}
    for j, i, w in taps:
        cols.setdefault(i, []).append((j, w))
    return int(any(len(ts) == 1 and ts[0][0] == jmin for ts in cols.values()))


_CACHE = {}


def _get_program(kern3: np.ndarray):
    """Pick the vertical orientation (normal / row-flipped) that lets a
    matmul column be pulled into the drain, then build + compile."""
    key = kern3.astype(np.float32).tobytes()
    if key not in _CACHE:
        flip = _pull_quality(kern3[::-1]) > _pull_quality(kern3)
        kern_o = kern3[::-1] if flip else kern3
        nc, meta = build_program(np.ascontiguousarray(kern_o))
        _CACHE[key] = (nc, meta, flip)
    return _CACHE[key]


def kernel(image: np.ndarray, kernel: np.ndarray) -> np.ndarray:
    image = np.asarray(image)
    kernel = np.asarray(kernel, dtype=np.float32)
    img = image.reshape(H, W)

    if not np.any(kernel):
        return np.zeros_like(image, dtype=np.float32).reshape(image.shape)

    nc, meta, flip = _get_program(kernel)

    padded = np.zeros((H + 2, W + 2), dtype=NP_BF16)
    padded[1:-1, 1:-1] = img  # cast fp32 -> fp16
    if flip:
        padded = padded[::-1]  # row-flip view; corr with flipped kernel
    in_maps = []
    for c in range(N_CORES):
        m = {"shard": np.ascontiguousarray(padded[c * RPC: c * RPC + RPC + 2])}
        if meta["bands"] is not None:
            m["bands"] = meta["bands"]
        in_maps.append(m)

    res = bass_utils.run_bass_kernel_spmd(nc, in_maps, core_ids=list(range(N_CORES)))
    out = np.concatenate([r["out"] for r in res.results], axis=0)
    if flip:
        out = out[::-1]
    return np.ascontiguousarray(out).astype(np.float32).reshape(image.shape)


# revision 30
# speedup vs baseline: 1.0755x; 1.0166x over previous
"""Trainium2 Bass kernel: 3x3 'same' cross-correlation on a [1,1,8192,8192] fp32 image.

Strategy (8 NeuronCores, row-sharded, memory-bound target):
  - Host: pad image by 1 row/col, cast to bf16 (tolerance is 2e-2 rel; bf16
    I/O costs ~4e-3), shard into 8 overlapping [1026, 8194] row-shards. The
    image and kernel are flipped vertically when that makes the single-tap
    kernel column's tap land on jmin (legal partition-0 pull, see below).
  - Device (per core): for each tile of R=126 output rows, one [R+nb, 8194]
    bf16 image tile A. Vertical taps are applied with one banded matmul per
    nonzero kernel *column* (PE shifts across partitions; free-dim offsets
    handle the kernel column on the rhs AP; bf16 streams 1 row/cycle vs
    fp16's 1/2). A kernel column holding a single tap at j==jmin is pulled
    out of the PE entirely and fused into the PSUM->SBUF drain as
    scalar_tensor_tensor (out = A_shifted * w + psum) — cuts PE work from 3
    to 2 matmuls per chunk. Drains split 3:1 between DVE (fused STT from
    PSUM) and ACT-copy + GPSIMD-add; output DMAs go out on the ACT HWDGE
    queue, input DMAs on SP.
  - bf16 halves HBM traffic both ways: ~16.8 MB in + 16.8 MB out per core,
    ~94us DMA roofline at 358 GB/s per-core HBM bandwidth.
  - Host: concat shard outputs, unflip, upcast to fp32.
"""

import ml_dtypes
import numpy as np

import concourse.bass as bass
import concourse.mybir as mybir
from concourse import bacc
from concourse import bass_utils
from concourse import tile

H = 8192
W = 8192
N_CORES = 8
RPC = H // N_CORES  # rows per core

BF16 = mybir.dt.bfloat16
NP_BF16 = ml_dtypes.bfloat16
F32 = mybir.dt.float32


def _nonzero_taps(kern3: np.ndarray):
    """[(j, i, w)] for nonzero entries of the 3x3 kernel."""
    return [
        (j, i, float(kern3[j, i]))
        for j in range(kern3.shape[0])
        for i in range(kern3.shape[1])
        if kern3[j, i] != 0.0
    ]


def _band_matrix(col_taps, jmin, k_rows, out_rows):
    """lhsT [k_rows, out_rows] with B[k, p] = w for each (j, w) in col_taps
    where k = p + (j - jmin). matmul computes psum[p,:] = sum_k B[k,p]*A[k,:]."""
    B = np.zeros((k_rows, out_rows), dtype=NP_BF16)
    for j, w in col_taps:
        d = j - jmin
        for p in range(out_rows):
            k = p + d
            if 0 <= k < k_rows:
                B[k, p] = w
    return B


def build_program(kern3: np.ndarray, *, width=W, rpc=RPC,
                  psum_cols=1024, mm_cols=512, a_bufs=4,
                  out_bufs=3, psum_bufs=4, tmp_bufs=4,
                  drain_paths=("dve", "actg", "dve", "act3")):
    """Build the per-core Bass program. Shard layout: S[r] =
    padded_image[core_row0 + r], r in [0, rpc+2); out rows r in [0, rpc).

    The image tile carries one zero column of padding each side so every
    tap's rhs slice [c0+i, c0+i+mm_cols) is in range and full-width."""
    taps = _nonzero_taps(kern3)
    assert taps, "all-zero kernel handled host-side"

    jmin = min(j for j, _, _ in taps)
    jmax = max(j for j, _, _ in taps)
    nb = jmax - jmin  # extra rows of A needed beyond R
    R = (128 - nb) & ~1  # output rows per tile (even)

    # group taps by kernel column
    cols = {}
    for j, i, w in taps:
        cols.setdefault(i, []).append((j, w))
    all_col_ids = sorted(cols.keys())

    # Pull one tap into the DVE drain when it erases a whole matmul column.
    # Engines can only address SBUF starting at partition 0, so the pulled
    # tap must sit at j == jmin (partition-aligned with the out tile).
    # ACT-drained chunks can't fuse the tap (no scalar_tensor_tensor on
    # ACT), so they run all three bands on the PE instead.
    pulled = None
    single = sorted(i for i, ts in cols.items()
                    if len(ts) == 1 and ts[0][0] == jmin)
    if single:
        pi = single[0]
        pj, pw = cols[pi][0]
        pulled = (pj, pi, pw)
    col_ids = sorted(i for i in cols if pulled is None or i != pulled[1])

    nc = bacc.Bacc("TRN2", target_bir_lowering=False, debug=False,
                   num_devices=N_CORES)
    s_in = nc.dram_tensor("shard", [rpc + 2, width + 2], BF16, kind="ExternalInput").ap()
    out_d = nc.dram_tensor("out", [rpc, width], BF16, kind="ExternalOutput").ap()
    bands_in = None
    if col_ids:
        # one [128, ncols*128] tensor: a single well-shaped DMA instead of
        # several 256B-per-partition ones that would clog the input queue
        bands_in = nc.dram_tensor(
            "bands", [128, len(all_col_ids) * 128], BF16, kind="ExternalInput"
        ).ap()

    # tiles of output rows; a short first tile lets compute start before a
    # full-height image load would finish (shorter pipeline fill), and the
    # natural remainder tile at the end shortens the drain->DMA tail.
    tiles = []
    t = 0
    first = min(64, rpc)
    tiles.append((0, first))
    t = first
    while t < rpc:
        r = min(R, rpc - t)
        tiles.append((t, r))
        t += r

    wp = width + 2  # padded tile width
    out_dma_eng = nc.scalar if mybir.EngineType.Activation in nc.hwdge_engines else nc.sync

    with tile.TileContext(nc) as tc:
        with (
            tc.tile_pool(name="bandp", bufs=1) as bandp,
            tc.tile_pool(name="ap", bufs=a_bufs) as apool,
            tc.tile_pool(name="op", bufs=out_bufs) as opool,
            tc.tile_pool(name="tp", bufs=tmp_bufs) as tpool,
            tc.tile_pool(name="pp", bufs=psum_bufs, space="PSUM") as ppool,
        ):
            band_tiles = {}
            if col_ids:
                ball = bandp.tile([128, len(all_col_ids) * 128], BF16,
                                  tag="bands")
                # off the SP queue so tile 0's image load starts immediately
                out_dma_eng.dma_start(out=ball, in_=bands_in)
                for ci, i in enumerate(all_col_ids):
                    band_tiles[i] = ball[:, ci * 128:(ci + 1) * 128]

            n_q = (width + psum_cols - 1) // psum_cols
            eng_i = 0
            half = (width // 2) + 2  # column split point for the input load
            for (t0, rt) in tiles:
                krows = rt + nb  # contraction rows for this tile
                a = apool.tile([128, wp], BF16, tag="a")
                # two column-half loads: chunks in the left half of the tile
                # only wait on the first one (finer DMA/compute interleave)
                nc.sync.dma_start(
                    out=a[0:krows, 0:half],
                    in_=s_in[t0 + jmin: t0 + jmin + krows, 0:half],
                )
                nc.sync.dma_start(
                    out=a[0:krows, half:wp],
                    in_=s_in[t0 + jmin: t0 + jmin + krows, half:wp],
                )
                o = opool.tile([128, width], BF16, tag="o")

                for q in range(n_q):
                    q0 = q * psum_cols
                    q1 = min(q0 + psum_cols, width)
                    path = drain_paths[eng_i % len(drain_paths)]
                    eng_i += 1
                    # Three chunk flavors, spread so every drain engine
                    # retires its chunks faster than the PE produces them
                    # (a PE stall drops it out of the 2.4 GHz p-state):
                    #   dve:  2 bands + DVE STT drain fusing the pulled tap
                    #   act3: all 3 bands + plain ACT copy drain
                    #   actg: 2 bands + ACT copy to tmp + GPSIMD tap add
                    if pulled is None or (path == "actg" and pulled[2] != 1.0):
                        path = "act3" if path != "dve" else "dve"
                    fuse = pulled is not None and path == "dve"
                    skip_pull = fuse or (pulled is not None and path == "actg")
                    mm_cols_set = col_ids if skip_pull else all_col_ids
                    if mm_cols_set:
                        ps = ppool.tile([128, psum_cols], F32, tag="ps")
                        # band-major: the PE reloads weights once per band
                        # per chunk. Pieces sit in distinct PSUM banks, so
                        # the interleaved accumulation groups are
                        # address-disjoint.
                        for ii, i in enumerate(mm_cols_set):
                            for c0 in range(q0, q1, mm_cols):
                                c1 = min(c0 + mm_cols, q1)
                                # rhs cols [c0+i, c1+i) in padded coords
                                nc.tensor.matmul(
                                    out=ps[0:rt, c0 - q0:c1 - q0],
                                    lhsT=band_tiles[i][0:krows, 0:rt],
                                    rhs=a[0:krows, c0 + i:c1 + i],
                                    start=(ii == 0),
                                    stop=(ii == len(mm_cols_set) - 1),
                                    skip_group_check=True,
                                )
                        if fuse:
                            # out[p, x] = A[p+pj, x+pi]*pw + psum[p, x]
                            pj, pi, pw = pulled
                            nc.vector.scalar_tensor_tensor(
                                out=o[0:rt, q0:q1],
                                in0=a[pj:pj + rt, q0 + pi:q1 + pi],
                                scalar=pw,
                                in1=ps[0:rt, 0:q1 - q0],
                                op0=mybir.AluOpType.mult,
                                op1=mybir.AluOpType.add,
                            )
                        elif path == "dve":
                            nc.vector.tensor_scalar_mul(
                                o[0:rt, q0:q1], ps[0:rt, 0:q1 - q0], 1.0
                            )
                        elif path == "actg":
                            pj, pi, pw = pulled
                            tmp = tpool.tile([128, psum_cols], BF16, tag="tmp")
                            nc.scalar.copy(
                                out=tmp[0:rt, 0:q1 - q0], in_=ps[0:rt, 0:q1 - q0]
                            )
                            nc.gpsimd.tensor_add(
                                o[0:rt, q0:q1],
                                a[pj:pj + rt, q0 + pi:q1 + pi],
                                tmp[0:rt, 0:q1 - q0],
                            )
                        else:
                            nc.scalar.copy(
                                out=o[0:rt, q0:q1], in_=ps[0:rt, 0:q1 - q0]
                            )
                    else:
                        # single-tap kernel: no matmul at all
                        pj, pi, pw = pulled
                        eng = nc.vector if path == "dve" else nc.gpsimd
                        eng.tensor_scalar_mul(
                            o[0:rt, q0:q1], a[pj:pj + rt, q0 + pi:q1 + pi], pw
                        )
                    # store each output column-half as soon as its chunks
                    # have drained — halves the drain->DMA tail latency
                    if q == n_q // 2 - 1:
                        out_dma_eng.dma_start(
                            out=out_d[t0: t0 + rt, 0:q1], in_=o[0:rt, 0:q1]
                        )
                    elif q == n_q - 1:
                        h0 = (n_q // 2) * psum_cols
                        out_dma_eng.dma_start(
                            out=out_d[t0: t0 + rt, h0:q1], in_=o[0:rt, h0:q1]
                        )

    nc.compile()

    meta = {
        "bands": (
            np.concatenate([
                _band_matrix(cols[i], jmin, 128, 128) for i in all_col_ids
            ], axis=1) if col_ids else None
        ),
    }
    return nc, meta


def _pull_quality(kern3: np.ndarray) -> int:
    """1 if this orientation admits a legal pulled tap (a single-tap kernel
    column whose tap sits at jmin), else 0."""
    taps = _nonzero_taps(kern3)
    if not taps:
        return 0
    jmin = min(j for j, _, _ in taps)
    cols = {}
    for j, i, w in taps:
        cols.setdefault(i, []).append((j, w))
    return int(any(len(ts) == 1 and ts[0][0] == jmin for ts in cols.values()))


_CACHE = {}


def _get_program(kern3: np.ndarray):
    """Pick the vertical orientation (normal / row-flipped) that lets a
    matmul column be pulled into the drain, then build + compile."""
    key = kern3.astype(np.float32).tobytes()
    if key not in _CACHE:
        flip = _pull_quality(kern3[::-1]) > _pull_quality(kern3)
        kern_o = kern3[::-1] if flip else kern3
        nc, meta = build_program(np.ascontiguousarray(kern_o))
        _CACHE[key] = (nc, meta, flip)
    return _CACHE[key]


def kernel(image: np.ndarray, kernel: np.ndarray) -> np.ndarray:
    image = np.asarray(image)
    kernel = np.asarray(kernel, dtype=np.float32)
    img = image.reshape(H, W)

    if not np.any(kernel):
        return np.zeros_like(image, dtype=np.float32).reshape(image.shape)

    nc, meta, flip = _get_program(kernel)

    padded = np.zeros((H + 2, W + 2), dtype=NP_BF16)
    padded[1:-1, 1:-1] = img  # cast fp32 -> fp16
    if flip:
        padded = padded[::-1]  # row-flip view; corr with flipped kernel
    in_maps = []
    for c in range(N_CORES):
        m = {"shard": np.ascontiguousarray(padded[c * RPC: c * RPC + RPC + 2])}
        if meta["bands"] is not None:
            m["bands"] = meta["bands"]
        in_maps.append(m)

    res = bass_utils.run_bass_kernel_spmd(nc, in_maps, core_ids=list(range(N_CORES)))
    out = np.concatenate([r["out"] for r in res.results], axis=0)
    if flip:
        out = out[::-1]
    return np.ascontiguousarray(out).astype(np.float32).reshape(image.shape)
